# revision 12
# baseline (speedup 1.0000x reference)
"""APPNP (nn_APPNPNet) on 8 TRN2 NeuronCores.

Self-contained kernel: kernel(**inputs) -> [100000, 64] float32.

Approach: nodes assigned to cores by a window-balancing greedy (each node's
in-edges spread evenly over the 4 replica windows = core-pairs), then
degree-ordered into (slot, lane) cells per core. MLP encoder in bf16
on-device. K=10 propagation steps, each: compact AllGather of dinv-scaled
features -> shared DRAM replica -> local expand to 256B-strided rows; per
(slot-group, window) a dma_gather pulls edge-source rows into slot-major
SBUF staging using SBUF-resident int16 indices; DVE folds slot pairs; PE
accumulates remaining slots into PSUM via identity matmul; fused epilogue
applies the GCN normalization and the alpha*h0 teleport term.
"""

import math
import sys
import types

import numpy as np

P = 128
FEAT = 64
ROWW = 128


def _install_profile_hook():
    try:
        from antenv import axon_hooks  # noqa: F401
        return
    except Exception:
        pass
    try:
        import antenv
        from trn_agent_boot.trn_boot import _ntff_profile_via_ctypes
        mod = types.ModuleType("antenv.axon_hooks")
        _hook = [None]
        mod.set_axon_ntff_profile_hook = lambda h: _hook.__setitem__(0, h)
        mod.get_axon_ntff_profile_hook = lambda: _hook[0]
        sys.modules["antenv.axon_hooks"] = mod
        antenv.axon_hooks = mod
        mod.set_axon_ntff_profile_hook(
            _ntff_profile_via_ctypes('/opt/axon/libaxon_pjrt.so'))
    except Exception:
        pass


# ------------------------------------------------------- core assignment
def assign_cores(ei, n_nodes, n_cores=8):
    """Assign nodes to cores so each dst node's in-edges spread evenly
    over the 4 windows (= source core-pairs). Batches of 8 consecutive
    in-degree ranks map to the 8 cores (preserving per-core degree
    profiles); within a batch a greedy picks windows to flatten each
    out-neighbor's per-window in-edge counts."""
    n_win = n_cores // 2
    src = np.concatenate([np.asarray(ei[0], np.int64),
                          np.arange(n_nodes, dtype=np.int64)])
    dst = np.concatenate([np.asarray(ei[1], np.int64),
                          np.arange(n_nodes, dtype=np.int64)])
    deg_in = np.bincount(dst, minlength=n_nodes)
    order = np.argsort(-deg_in, kind="stable")

    eorder = np.argsort(src, kind="stable")
    dst_sorted = dst[eorder].astype(np.int32)
    out_start = np.zeros(n_nodes + 1, np.int64)
    out_start[1:] = np.cumsum(np.bincount(src, minlength=n_nodes))

    cnt = np.zeros((n_nodes, n_win), np.int32)
    core_of = np.empty(n_nodes, np.int32)
    win_of = np.empty(n_nodes, np.int32)
    targ = ((deg_in + n_win - 1) // n_win).astype(np.int32)

    def solve_batch(batch, nb, costs):
        spread = costs.max(axis=1) - costs.min(axis=1)
        cap = np.full(n_win, 2, np.int32)
        for i in np.argsort(-spread):
            c = costs[i].copy()
            c[cap <= 0] = np.iinfo(np.int64).max
            w = int(np.argmin(c))
            cap[w] -= 1
            win_of[batch[i]] = w
            if len(nb[i]):
                cnt[nb[i], w] += 1

    for b0 in range(0, n_nodes, n_cores):
        batch = order[b0:b0 + n_cores]
        nb = [dst_sorted[out_start[s]:out_start[s + 1]] for s in batch]
        costs = np.zeros((len(batch), n_win), np.int64)
        for i in range(len(batch)):
            if len(nb[i]):
                costs[i] = cnt[nb[i]].sum(axis=0)
        solve_batch(batch, nb, costs)

    for _ in range(2):  # refinement: minimize per-dst overflow over target
        for b0 in range(0, n_nodes, n_cores):
            batch = order[b0:b0 + n_cores]
            nb = [dst_sorted[out_start[s]:out_start[s + 1]] for s in batch]
            for i in range(len(batch)):
                if len(nb[i]):
                    cnt[nb[i], win_of[batch[i]]] -= 1
            costs = np.zeros((len(batch), n_win), np.int64)
            for i in range(len(batch)):
                if len(nb[i]):
                    over = np.maximum(cnt[nb[i]] + 1 - targ[nb[i], None], 0)
                    costs[i] = (over.astype(np.int64).sum(axis=0) * 1024
                                + cnt[nb[i]].sum(axis=0))
            solve_batch(batch, nb, costs)

    # core within window: alternate to keep per-core counts exact
    slot_used = np.zeros((n_nodes // n_cores, n_win), np.int32)
    for b0 in range(0, n_nodes, n_cores):
        bi = b0 // n_cores
        for s in order[b0:b0 + n_cores]:
            w = win_of[s]
            core_of[s] = 2 * w + slot_used[bi, w]
            slot_used[bi, w] += 1
    return core_of, order, deg_in, cnt


# ---------------------------------------------------------------- host plan
def build_plan(ei, n_nodes, n_cores=8, g_blk=4, fold_levels=0, s_round=1):
    src = np.asarray(ei[0], dtype=np.int64)
    dst = np.asarray(ei[1], dtype=np.int64)
    loops = np.arange(n_nodes, dtype=np.int64)
    src = np.concatenate([src, loops])
    dst = np.concatenate([dst, loops])
    deg = np.bincount(dst, minlength=n_nodes).astype(np.int64)
    dinv = np.zeros(n_nodes, np.float64)
    nz = deg > 0
    dinv[nz] = 1.0 / np.sqrt(deg[nz])

    core_of, order, deg_in, wcnt = assign_cores(ei, n_nodes, n_cores)

    per_core = n_nodes // n_cores
    n_slots = math.ceil(per_core / P)
    SH = n_slots * P
    R = n_cores * SH
    WIN = 2 * SH
    n_win = n_cores // 2
    assert WIN <= 32768

    # rank within core by worst-window count (concentrates high-S nodes
    # into few slot-groups), degree as tiebreak
    nr = core_of
    badness = wcnt.max(axis=1).astype(np.int64) * 1024 + deg_in
    k_within = np.empty(n_nodes, np.int64)
    for r in range(n_cores):
        nodes_r = np.flatnonzero(nr == r)
        nodes_r = nodes_r[np.argsort(-badness[nodes_r], kind="stable")]
        k_within[nodes_r] = np.arange(len(nodes_r))
        assert len(nodes_r) <= SH
    nj = k_within // P
    np_lane = k_within % P
    nr = nr.astype(np.int64)

    repl_row = nr * SH + np_lane * n_slots + nj

    occupied = np.zeros(R, bool)
    occupied[repl_row] = True
    free_rows = np.flatnonzero(~occupied)
    zrow_of_win = np.zeros(n_win, np.int64)
    for w in range(n_win):
        cand = free_rows[(free_rows >= w * WIN) & (free_rows < (w + 1) * WIN)]
        assert len(cand) > 0, f"no zero row in window {w}"
        zrow_of_win[w] = cand[0]

    e_r = nr[dst]
    e_j = nj[dst]
    e_p = np_lane[dst]
    e_row = repl_row[src]
    e_w = e_row // WIN

    groups = []
    j = 0
    while j < n_slots:
        G = min(g_blk, n_slots - j)
        groups.append((j, G))
        j += G
    grp_of_slot = np.zeros(n_slots, np.int64)
    for gi, (j0, G) in enumerate(groups):
        grp_of_slot[j0:j0 + G] = gi
    n_grp = len(groups)

    lane_key = ((e_r * n_slots + e_j) * n_win + e_w) * P + e_p
    lane_cnt = np.bincount(lane_key, minlength=n_cores * n_slots * n_win * P)
    lane_cnt = lane_cnt.reshape(n_cores, n_slots, n_win, P)
    S = np.zeros((n_grp, n_win), np.int64)
    for gi, (j0, G) in enumerate(groups):
        m = lane_cnt[:, j0:j0 + G].max(axis=(0, 1, 3))
        S[gi] = np.maximum(((m + s_round - 1) // s_round) * s_round, s_round)

    zone_pos0 = np.zeros((n_grp, n_win), np.int64)
    pos = 0
    for gi, (j0, G) in enumerate(groups):
        for w in range(n_win):
            zone_pos0[gi, w] = pos
            pos += int(S[gi, w]) * G * P
    n_pos = pos

    skey = lane_key
    eorder = np.argsort(skey, kind="stable")
    inv_start = np.zeros(lane_cnt.size + 1, np.int64)
    inv_start[1:] = np.cumsum(lane_cnt.reshape(-1))
    s_in = np.arange(len(skey)) - inv_start[skey[eorder]]
    s_in_unsorted = np.empty_like(s_in)
    s_in_unsorted[eorder] = s_in
    e_b = e_j - np.array([j0 for (j0, G) in groups])[grp_of_slot[e_j]]
    e_gp = grp_of_slot[e_j]
    e_pos = (zone_pos0[e_gp, e_w]
             + (s_in_unsorted * np.array([G for (j0, G) in groups])[e_gp]
                + e_b) * P + e_p)

    pos_w = np.empty(n_pos, np.int64)
    for gi in range(n_grp):
        for w in range(n_win):
            a = zone_pos0[gi, w]
            G = groups[gi][1]
            pos_w[a:a + int(S[gi, w]) * G * P] = w
    idx16 = np.zeros((n_cores, P, n_pos // 16), np.int16)
    for r in range(n_cores):
        allrows = zrow_of_win[pos_w].copy()
        m = e_r == r
        allrows[e_pos[m]] = e_row[m]
        rel = allrows - pos_w * WIN
        assert rel.min() >= 0 and rel.max() < WIN
        wr = rel.reshape(-1, 16).T.astype(np.int16)
        idx16[r] = np.tile(wr, (8, 1))

    dinvT = np.zeros((n_cores, P, n_slots), np.float32)
    node_ids = np.arange(n_nodes)
    dinvT[nr[node_ids], np_lane[node_ids], nj[node_ids]] = \
        dinv[node_ids].astype(np.float32)

    return dict(
        n_nodes=n_nodes, n_cores=n_cores, n_slots=n_slots, SH=SH, R=R,
        WIN=WIN, n_win=n_win, groups=groups, S=S, zone_pos0=zone_pos0,
        n_pos=n_pos, idx16=idx16, dinvT=dinvT, zrow_of_win=zrow_of_win,
        pos_w=pos_w, nr=nr, np_lane=np_lane, nj=nj, fold_levels=fold_levels,
    )


def shard_inputs(plan, x):
    n_cores, SH = plan["n_cores"], plan["SH"]
    in_c = x.shape[1]
    xT = np.zeros((n_cores, in_c, SH), np.float32)
    node_ids = np.arange(plan["n_nodes"])
    cols = plan["nj"][node_ids] * P + plan["np_lane"][node_ids]
    xT[plan["nr"][node_ids], :, cols] = x[node_ids]
    return xT


def assemble_output(plan, core_outs):
    n_nodes, n_slots = plan["n_nodes"], plan["n_slots"]
    out = np.empty((n_nodes, FEAT), np.float32)
    node_ids = np.arange(n_nodes)
    rows = plan["np_lane"][node_ids] * n_slots + plan["nj"][node_ids]
    rs = plan["nr"][node_ids]
    for r in range(plan["n_cores"]):
        m = rs == r
        out[node_ids[m]] = core_outs[r][rows[m]]
    return out


# ----------------------------------------------------- custom gather emitter
def emit_dma_gather(nc, out_ap, in_ap, idxs_ap, num_idxs, elem_size,
                    elem_step, queue_num=0):
    import concourse.mybir as mybir
    from concourse import ap_utils
    from concourse.bass import round_up_to_multiple, exact_div

    eng = nc.gpsimd
    assert idxs_ap.dtype == mybir.dt.int16
    assert in_ap.dtype == out_ap.dtype
    assert ap_utils.ap_is_contiguous(in_ap.ap[1:])
    assert ap_utils.ap_is_contiguous(out_ap.ap[1:])
    assert ap_utils.ap_is_contiguous(idxs_ap.ap[1:])
    assert in_ap.ap[-1][1] == out_ap.ap[-1][1] == elem_size
    assert out_ap.ap[0][1] * out_ap.ap[1][1] == \
        round_up_to_multiple(num_idxs, 128)
    assert in_ap.ap[0][0] == elem_step
    stride_bytes = elem_step * mybir.dt.size(in_ap.dtype)
    stride_bytes_256 = exact_div(stride_bytes, 256)
    _in_ap = eng.lower_ap_dma(in_ap, for_custom_bir_dma=True)
    _idxs_ap = eng.lower_ap(idxs_ap)
    _out_ap = eng.lower_ap(out_ap)
    return eng.add_instruction(
        mybir.InstDMAGatherAnt(
            name=nc.get_next_instruction_name(),
            ins=[*_in_ap, _idxs_ap,
                 eng.lower_val_access(eng.to_reg(num_idxs))],
            outs=[_out_ap],
            num_idxs=num_idxs,
            elem_size=elem_size,
            stride_bytes_256=stride_bytes_256,
            transpose=False,
            gen_mode=0,
            single_packet=False,
            queue_num=queue_num,
            sbuf_tokens_per_rank=0,
            sbuf_free_dim_per_rank=0,
            sbuf_free_dim_pad_per_rank=0,
            sbuf_byte_offset=0,
        )
    )


def _fold_plan(S, fold_levels):
    """Number of identity-matmul columns after DVE folding for each S."""
    s = int(S)
    for _ in range(fold_levels):
        if s % 2:
            break
        s //= 2
    return s


# ------------------------------------------------------------- device graph
def build_graph(plan, in_c=512, hid=256, K=10):
    import concourse.mybir as mybir
    from concourse import bacc, tile
    from concourse.masks import make_identity

    n_cores, SH, n_slots = plan["n_cores"], plan["SH"], plan["n_slots"]
    n_win, S, R, WIN = plan["n_win"], plan["S"], plan["R"], plan["WIN"]
    n_pos = plan["n_pos"]
    fold_levels = plan["fold_levels"]
    bf, f32, i16 = mybir.dt.bfloat16, mybir.dt.float32, mybir.dt.int16
    AOT = mybir.AluOpType
    AFT = mybir.ActivationFunctionType
    KT1 = in_c // P
    HHALF = hid // P

    nc = bacc.Bacc("TRN2", target_bir_lowering=False, debug=False,
                   num_devices=n_cores, num_swdge_queues=4)
    xT_d = nc.dram_tensor("xT", [in_c, SH], bf, kind="ExternalInput")
    W1_d = nc.dram_tensor("W1", [in_c, hid], bf, kind="ExternalInput")
    b1_d = nc.dram_tensor("b1h", [P, HHALF], f32, kind="ExternalInput")
    W2_d = nc.dram_tensor("W2", [hid, FEAT], bf, kind="ExternalInput")
    b2_d = nc.dram_tensor("b2", [1, FEAT], bf, kind="ExternalInput")
    dinv_d = nc.dram_tensor("dinvT", [P, n_slots], f32, kind="ExternalInput")
    idx_d = nc.dram_tensor("idx", [P, n_pos // 16], i16, kind="ExternalInput")
    out_d = nc.dram_tensor("out", [P, n_slots, FEAT], f32,
                           kind="ExternalOutput")

    with tile.TileContext(nc) as tc:
        with tc.tile_pool(name="persist", bufs=1) as pers, \
             tc.tile_pool(name="dram", bufs=1, space="DRAM") as dramp:
            dinv_sb = pers.tile([P, n_slots], f32)
            nc.sync.dma_start(dinv_sb[:], dinv_d[:])
            b1_sb = pers.tile([P, HHALF], f32)
            nc.sync.dma_start(b1_sb[:], b1_d[:])
            b2_sb = pers.tile([1, FEAT], bf)
            nc.sync.dma_start(b2_sb[:], b2_d[:])
            d2_sb = pers.tile([P, n_slots], f32)
            d9_sb = pers.tile([P, n_slots], f32)
            d01_sb = pers.tile([P, n_slots], f32)
            nc.vector.scalar_tensor_tensor(
                out=d2_sb[:], in0=dinv_sb[:], scalar=0.9, in1=dinv_sb[:],
                op0=AOT.mult, op1=AOT.mult)
            nc.vector.tensor_scalar_mul(d9_sb[:], dinv_sb[:], 0.9)
            nc.vector.tensor_scalar_mul(d01_sb[:], dinv_sb[:], 0.1)
            ident = pers.tile([P, P], bf)
            make_identity(nc, ident[:])
            ones1 = pers.tile([1, P], bf)
            nc.vector.memset(ones1[:], 1.0)
            g0s_sb = pers.tile([P, n_slots * FEAT], bf)
            gnext_sb = pers.tile([P, n_slots * ROWW], bf)
            nc.vector.memset(gnext_sb[:], 0.0)

            g_in = dramp.tile([P, n_slots * ROWW], bf)
            replicas = [dramp.tile([R, ROWW], bf, addr_space="Shared",
                                   tag=f"replica{k}", name=f"replica{k}")
                        for k in range(K)]
            h0s_dram = dramp.tile([P, n_slots * FEAT], f32)

            # ---------------- MLP ----------------
            with tc.tile_pool(name="mlp_w", bufs=1) as wp, \
                 tc.tile_pool(name="mlp_x", bufs=2) as mpx, \
                 tc.tile_pool(name="mlp_big", bufs=1) as bigp, \
                 tc.tile_pool(name="mlp_ps1", bufs=2, space="PSUM") as pp1, \
                 tc.tile_pool(name="mlp_ps2", bufs=2, space="PSUM") as pp2:
                w1t = []
                for k in range(KT1):
                    t = wp.tile([P, hid], bf, tag=f"w1_{k}", name=f"w1_{k}")
                    nc.sync.dma_start(t[:], W1_d[k * P:(k + 1) * P, :])
                    w1t.append(t)
                w2t = []
                for k in range(HHALF):
                    t = wp.tile([P, FEAT], bf, tag=f"w2_{k}", name=f"w2_{k}")
                    nc.sync.dma_start(t[:], W2_d[k * P:(k + 1) * P, :])
                    w2t.append(t)
                h1T = [bigp.tile([P, SH], bf, tag=f"h1T{h}", name=f"h1T{h}")
                       for h in range(HHALF)]
                h0s_stage = bigp.tile([P, n_slots * FEAT], f32, tag="h0s")
                CH = 512
                nch = (SH + CH - 1) // CH
                for c in range(nch):
                    c0 = c * CH
                    cw = min(CH, SH - c0)
                    xts = []
                    for k in range(KT1):
                        t = mpx.tile([P, CH], bf, tag=f"xt{k}", name=f"xt{k}")
                        nc.sync.dma_start(t[:, :cw],
                                          xT_d[k * P:(k + 1) * P, c0:c0 + cw])
                        xts.append(t)
                    for h in range(HHALF):
                        ps = pp1.tile([P, CH], f32, tag="ps1")
                        for k in range(KT1):
                            nc.tensor.matmul(
                                ps[:, :cw], w1t[k][:, h * P:(h + 1) * P],
                                xts[k][:, :cw], start=(k == 0),
                                stop=(k == KT1 - 1))
                        nc.scalar.activation(
                            out=h1T[h][:, c0:c0 + cw], in_=ps[:, :cw],
                            func=AFT.Relu, bias=b1_sb[:, h:h + 1], scale=1.0)
                for j in range(n_slots):
                    ps2 = pp2.tile([P, FEAT], f32, tag="ps2")
                    for k in range(HHALF):
                        nc.tensor.matmul(ps2[:], h1T[k][:, j * P:(j + 1) * P],
                                         w2t[k][:], start=(k == 0), stop=False)
                    nc.tensor.matmul(ps2[:], ones1[:], b2_sb[:],
                                     start=False, stop=True)
                    slf = slice(j * FEAT, (j + 1) * FEAT)
                    slg = slice(j * ROWW, j * ROWW + FEAT)
                    nc.scalar.mul(h0s_stage[:, slf], ps2[:], 0.1)
                    nc.vector.tensor_scalar_mul(gnext_sb[:, slg], ps2[:],
                                                dinv_sb[:, j:j + 1])
                    nc.vector.tensor_scalar_mul(g0s_sb[:, slf], ps2[:],
                                                d01_sb[:, j:j + 1])
                nc.sync.dma_start(h0s_dram[:], h0s_stage[:])

            # ---------------- propagation ----------------
            with tc.tile_pool(name="idxr", bufs=1) as idxrp, \
                 tc.tile_pool(name="zone", bufs=4) as zonep, \
                 tc.tile_pool(name="tmps", bufs=2) as tmpp, \
                 tc.tile_pool(name="outst", bufs=2) as outp, \
                 tc.tile_pool(name="h0sl", bufs=2) as h0slp, \
                 tc.tile_pool(name="lps", bufs=2, space="PSUM") as lpsp:
                idx_sb = idxrp.tile([P, n_pos // 16], i16)
                nc.sync.dma_start(idx_sb[:], idx_d[:])
                qrot = [0]
                for k in range(K):
                    replica = replicas[k]
                    nc.sync.dma_start(g_in[:], gnext_sb[:])
                    nc.gpsimd.collective_compute(
                        "AllGather", AOT.bypass,
                        replica_groups=[list(range(n_cores))],
                        ins=[g_in.opt()], outs=[replica.opt()])
                    last = (k == K - 1)
                    for gi, (j0, G) in enumerate(plan["groups"]):
                        W = G * FEAT
                        if last:
                            h0s_sb = h0slp.tile([P, W], f32, tag="h0c",
                                                name="h0c")
                            nc.sync.dma_start(
                                h0s_sb[:],
                                h0s_dram[:, j0 * FEAT:(j0 + G) * FEAT])
                        ps = lpsp.tile([P, W], f32, tag="lps")
                        first_mm = True
                        n_mm = sum(int(S[gi, w]) for w in range(n_win))
                        mm_done = 0
                        for w in range(n_win):
                            Sw = int(S[gi, w])
                            npos_z = Sw * G * P
                            zt = zonep.tile([P, Sw * W], bf, tag="zone",
                                            name="zt")
                            w0 = w * WIN
                            wrows = min(WIN, R - w0)
                            c0 = plan["zone_pos0"][gi, w] // 16
                            CAP = 8192
                            ztv = zt[:].rearrange("p (c f) -> p c f",
                                                  c=Sw * G, f=FEAT)
                            for q0 in range(0, npos_z, CAP):
                                nq = min(CAP, npos_z - q0)
                                emit_dma_gather(
                                    nc,
                                    out_ap=ztv[:, q0 // P:(q0 + nq) // P, :],
                                    in_ap=replica[w0:w0 + wrows, 0:FEAT],
                                    idxs_ap=idx_sb[
                                        :, c0 + q0 // 16:c0 + (q0 + nq) // 16],
                                    num_idxs=nq, elem_size=FEAT,
                                    elem_step=ROWW,
                                    queue_num=qrot[0] % 4)
                                qrot[0] += 1
                            for t in range(Sw):
                                nc.tensor.matmul(
                                    ps[:], ident[:],
                                    zt[:, t * W:(t + 1) * W],
                                    start=first_mm,
                                    stop=(mm_done == n_mm - 1))
                                first_mm = False
                                mm_done += 1
                        if last:
                            ot = outp.tile([P, W], f32, tag="out", name="ot")
                        for b in range(G):
                            j = j0 + b
                            slr = slice(b * FEAT, (b + 1) * FEAT)
                            slf = slice(j * FEAT, (j + 1) * FEAT)
                            slg = slice(j * ROWW, j * ROWW + FEAT)
                            if not last:
                                nc.vector.scalar_tensor_tensor(
                                    out=gnext_sb[:, slg], in0=ps[:, slr],
                                    scalar=d2_sb[:, j:j + 1],
                                    in1=g0s_sb[:, slf],
                                    op0=AOT.mult, op1=AOT.add)
                            else:
                                nc.vector.scalar_tensor_tensor(
                                    out=ot[:, slr], in0=ps[:, slr],
                                    scalar=d9_sb[:, j:j + 1],
                                    in1=h0s_sb[:, slr],
                                    op0=AOT.mult, op1=AOT.add)
                        if last:
                            nc.sync.dma_start(
                                out_d[:, j0:j0 + G, :],
                                ot[:].rearrange("p (j f) -> p j f",
                                                j=G, f=FEAT))
    nc.compile()
    return nc


def _exec_ns_from_ntff(tmpdir):
    """Fallback: extract exec time by converting the saved NTFF manually."""
    import glob
    import json
    import os
    import subprocess
    try:
        ntffs = glob.glob(os.path.join(tmpdir, "*_body*.ntff"))
        neffs = glob.glob(os.path.join(tmpdir, "*_body*.neff"))
        if not ntffs or not neffs:
            return None
        jpath = os.path.join(tmpdir, "manual_ntff.json")
        subprocess.check_call(
            ["neuron-profile", "view", "--ignore-nc-buf-usage",
             "-s", os.path.basename(ntffs[0]),
             "-n", os.path.basename(neffs[0]),
             "--output-format=json", f"--output-file={jpath}",
             "--ignore-dma-trace"],
            cwd=tmpdir)
        with open(jpath) as f:
            summ = json.load(f)["summary"][0]
        return int(round(float(summ["total_time"]) * 1e9))
    except Exception:
        return None


def kernel(x, ei, W1, b1, W2, b2):
    for pth in ('/opt/trn_rl_repo', '/root/.axon_site/_ro/trn_rl_repo'):
        if pth not in sys.path:
            sys.path.insert(0, pth)
    _install_profile_hook()
    import ml_dtypes
    from concourse.bass_utils import run_bass_kernel_spmd

    bf16 = ml_dtypes.bfloat16
    x = np.asarray(x, np.float32)
    ei = np.asarray(ei)
    W1 = np.asarray(W1, np.float32)
    b1 = np.asarray(b1, np.float32)
    W2 = np.asarray(W2, np.float32)
    b2 = np.asarray(b2, np.float32)
    n_nodes, in_c = x.shape
    hid = W1.shape[1]
    n_cores = 8

    plan = build_plan(ei, n_nodes, n_cores=n_cores)
    nc = build_graph(plan, in_c=in_c, hid=hid, K=10)

    xT = shard_inputs(plan, x)
    in_maps = []
    for r in range(n_cores):
        in_maps.append({
            "xT": xT[r].astype(bf16),
            "W1": W1.astype(bf16),
            "b1h": np.ascontiguousarray(
                b1.reshape(hid // P, P).T.astype(np.float32)),
            "W2": W2.astype(bf16),
            "b2": b2.reshape(1, FEAT).astype(bf16),
            "dinvT": np.ascontiguousarray(plan["dinvT"][r]),
            "idx": np.ascontiguousarray(plan["idx16"][r]),
        })
    import os
    trace = bool(os.environ.get("APPNP_TRACE"))
    tmpdir = None
    if trace:
        import tempfile
        tmpdir = os.environ.get("APPNP_TRACE_DIR") or tempfile.mkdtemp()
        os.makedirs(tmpdir, exist_ok=True)
        print(f"trace dir: {tmpdir}", flush=True)
    try:
        res = run_bass_kernel_spmd(nc, in_maps, core_ids=list(range(n_cores)),
                                   trace=trace, tmpdir=tmpdir)
        exec_ns = res.exec_time_ns
    except Exception as e:
        print(f"traced run post-processing failed ({e!r}); "
              f"retrying without trace", flush=True)
        res = run_bass_kernel_spmd(nc, in_maps, core_ids=list(range(n_cores)),
                                   trace=False)
        exec_ns = _exec_ns_from_ntff(tmpdir)
    if trace and exec_ns is not None:
        print(f"HW exec time: {exec_ns} ns")
    core_outs = [res.results[r]["out"].reshape(P * plan["n_slots"], FEAT)
                 for r in range(n_cores)]
    return assemble_output(plan, core_outs)

